# revision 15
# baseline (speedup 1.0000x reference)
"""CRF sequence-score kernel for Trainium2 (8 NeuronCores, SPMD).

Strategy (S-shard: core k owns s in [64k, 64k+64), all 512 batches):
  rows r = s_local*512 + b, laid out as [q = r%128 partitions, x = r//128].
  - emit[r] = emissions[r, tags[r]] via ONE indirect-DMA gather from the
    flat emissions shard: idx = 128*r + tag[r].
  - trans[r] = T[tag_r, tagnext_r] via a second indirect-DMA gather from
    the flat f32 transition table: idx = 128*tag_r + tagnext_r.
  - masks folded in a small epilogue; reduction over s via AP-strided
    tensor_reduce; start/end terms via [128,4] indirect gathers (end term
    exact: mask column-sum -> last tag gather -> end table gather).
Host sums the 8 per-core [128, 4] partials; score[b] = total[b%128, b//128].
"""
import numpy as np

SEQ, BATCH, NTAGS = 512, 512, 128
NCORES = 8
SLICE = SEQ // NCORES            # 64 s-rows per core
NROWS = SLICE * BATCH            # 32768 rows per core
P = 128
NX = NROWS // P                  # 256 columns in [q, x] layout
GCHUNK = 128                     # columns per indirect-gather instruction

_RUNNER = None


# ---------------------------------------------------------------------------
# walrus workaround: this build allows only ONE sync-wait per instruction.
def _install_tile_patch():
    import bass_rust
    import concourse.mybir as mybir
    import concourse.tile as tile
    from concourse.vector_clock import ScopedClock

    if getattr(tile.TileContext, "_crf_patched", False):
        return

    def _drain_and_barrier(self, tick_clock, wait_clock):
        nc = self.nc
        drain_inst = nc.sync.drain()
        wait_clock.add_sem_waits(
            drain_inst.ins, ScopedClock({None: tick_clock.global_clock})
        )
        si = drain_inst.ins.sync_info
        waits = list(si.on_wait) if si is not None and si.on_wait else []
        if len(waits) > 1:
            si.on_wait = waits[:1]
            for w in waits[1:]:
                extra = nc.sync.drain()
                if extra.ins.sync_info is None:
                    extra.ins.sync_info = bass_rust.SyncInfo(on_wait=[], on_update=[])
                extra.ins.sync_info.on_wait = [w]
        nc.all_engine_barrier()
        assert self.sems is not None
        popped = nc._tile_sem_poison_stack.pop()
        assert popped is self._sem_poison
        nc.clear_and_free_semaphores(list(self.sems.allocated().values()))
        nc.all_engine_barrier()

    orig_commit = tile.TileContext._commit_instruction

    def _commit(self, inst, lazy_reg_writes=True):
        si = getattr(inst, "sync_info", None)
        if (
            si is not None
            and si.on_wait
            and len(si.on_wait) > 1
            and inst.engine != mybir.EngineType.Unassigned
        ):
            waits = list(si.on_wait)
            si.on_wait = waits[:1]
            for w in waits[1:]:
                nop = mybir.InstNoOp(name=f"I-{self.nc.next_id()}", ins=[], outs=[])
                nop.engine = inst.engine
                nop.sync_info = bass_rust.SyncInfo(on_wait=[w], on_update=[])
                self._add_instruction(nop)
        return orig_commit(self, inst, lazy_reg_writes)

    tile.TileContext._drain_and_barrier = _drain_and_barrier
    tile.TileContext._commit_instruction = _commit
    tile.TileContext._crf_patched = True


# ---------------------------------------------------------------------------
def _build_nc():
    import concourse.bass as bass
    import concourse.mybir as mybir
    import concourse.tile as tile
    from concourse.masks import make_identity

    F32, I32, BF16 = mybir.dt.float32, mybir.dt.int32, mybir.dt.bfloat16
    AL = mybir.AluOpType

    nc = bass.Bass()
    em = nc.declare_dram_parameter("em", [1, NROWS * NTAGS], F32, isOutput=False)
    tagx_i = nc.declare_dram_parameter("tagx_i", [NROWS], I32, isOutput=False)
    tagnx_i = nc.declare_dram_parameter("tagnx_i", [NROWS], I32, isOutput=False)
    maskem_i = nc.declare_dram_parameter("maskem_i", [NROWS], I32, isOutput=False)
    masktr_i = nc.declare_dram_parameter("masktr_i", [NROWS], I32, isOutput=False)
    tmatf = nc.declare_dram_parameter("tmatf", [1, NTAGS * NTAGS], F32, isOutput=False)
    startv = nc.declare_dram_parameter("startv", [1, NTAGS], F32, isOutput=False)
    endv = nc.declare_dram_parameter("endv", [1, NTAGS], F32, isOutput=False)
    maskt_bf = nc.declare_dram_parameter("maskt_bf", [SEQ * BATCH], BF16, isOutput=False)
    tagf_i = nc.declare_dram_parameter("tagf_i", [1, SEQ * BATCH], I32, isOutput=False)
    out = nc.declare_dram_parameter("out", [P, 4], F32, isOutput=True)

    with tile.TileContext(nc) as tc:
        with tc.tile_pool(name="sbuf", bufs=1) as sb, \
             tc.tile_pool(name="psum", bufs=1, space="PSUM") as ps:
            # ---- loads: tags first (gate the gathers), then full bf16 mask
            # raw[p, s2*128+i] = v[s2*16384 + p*128 + i]
            def load_raw(name, dram):
                raw = sb.tile([P, NX], I32, name=f"{name}_raw")
                nc.sync.dma_start(
                    out=raw[:].rearrange("p (s i) -> p s i", s=2),
                    in_=dram[:].rearrange("(s p i) -> p s i", s=2, p=P, i=P),
                )
                return raw

            tagx_r = load_raw("tagt", tagx_i)
            tagnx_r = load_raw("tagnt", tagnx_i)
            # mt2[q, j*512+s] = mask[s, b=128j+q]  (host-transposed bf16)
            mt2 = sb.tile([P, 4 * 512], BF16, name="mt2")
            nc.sync.dma_start(
                out=mt2[:].rearrange("q (j s) -> q j s", j=4),
                in_=maskt_bf[:].rearrange("(j q s) -> q j s", j=4, q=P, s=512),
            )
            mex_r = load_raw("memt", maskem_i)
            mtx_r = load_raw("mtrt", masktr_i)

            # ---- constants
            ident = sb.tile([P, P], F32, name="ident")
            make_identity(nc, ident[:])
            # A[q, x] = 16384*x + 128*q  (flat-row offset of row r = 128x+q)
            a_i = sb.tile([P, NX], I32, name="a_i")
            nc.gpsimd.iota(a_i[:], pattern=[[NTAGS * P, NX]], base=0,
                           channel_multiplier=NTAGS)
            a_f = sb.tile([P, NX], F32, name="a_f")
            nc.gpsimd.tensor_copy(out=a_f[:], in_=a_i[:])
            iop_i = sb.tile([P, 1], I32, name="iop_i")
            nc.gpsimd.iota(iop_i[:], pattern=[[0, 1]], base=0, channel_multiplier=1)
            iop = sb.tile([P, 1], F32, name="iop")
            nc.gpsimd.tensor_copy(out=iop[:], in_=iop_i[:])
            j128_i = sb.tile([P, 4], I32, name="j128_i")
            nc.gpsimd.iota(j128_i[:], pattern=[[P, 4]], base=0, channel_multiplier=0)
            j128 = sb.tile([P, 4], F32, name="j128")
            nc.gpsimd.tensor_copy(out=j128[:], in_=j128_i[:])

            # ---- staging: i32 -> f32 -> PE transpose to [q, x] layout
            # t[q, 128h+m] = v[128*(128h+m)+q]
            def to_qx(name, raw, keep_sbuf=True, tag_pfx="tb"):
                f = sb.tile([P, NX], F32, name=f"{name}_f")
                nc.vector.tensor_copy(out=f[:], in_=raw[:])
                t = sb.tile([P, NX], F32, name=f"{name}_t") if keep_sbuf else None
                tps = []
                for h in range(2):
                    tp = ps.tile([P, P], F32, name=f"{name}_tp{h}", tag=f"{tag_pfx}{h}")
                    nc.tensor.transpose(out=tp[:], in_=f[:, h * P:(h + 1) * P],
                                        identity=ident[:])
                    tps.append(tp)
                    if keep_sbuf:
                        nc.scalar.copy(out=t[:, h * P:(h + 1) * P], in_=tp[:])
                return t, tps

            emitv = sb.tile([P, NX], F32, name="emitv")
            transv = sb.tile([P, NX], F32, name="transv")
            idx_em = sb.tile([P, NX], I32, name="idx_em")
            idx_tr = sb.tile([P, NX], I32, name="idx_tr")
            _, tag_tps = to_qx("tagt", tagx_r, keep_sbuf=False, tag_pfx="ta")
            _, tagn_tps = to_qx("tagnt", tagnx_r, keep_sbuf=False, tag_pfx="tn")
            # gather indices straight from PSUM transposes
            idx_em_f = sb.tile([P, NX], F32, name="idx_em_f")
            idx_tr_f = sb.tile([P, NX], F32, name="idx_tr_f")
            sidx = sb.tile([P, 4], I32, name="sidx")
            nc.vector.tensor_copy(out=sidx[:], in_=tag_tps[0][:, 0:4])
            for h in range(2):
                hs = slice(h * P, (h + 1) * P)
                nc.vector.tensor_tensor(out=idx_em_f[:, hs], in0=a_f[:, hs],
                                        in1=tag_tps[h][:], op=AL.add)
                nc.vector.tensor_copy(out=idx_em[:, hs], in_=idx_em_f[:, hs])
                nc.vector.scalar_tensor_tensor(
                    out=idx_tr_f[:, hs], in0=tag_tps[h][:], scalar=float(NTAGS),
                    in1=tagn_tps[h][:], op0=AL.mult, op1=AL.add,
                )
                nc.vector.tensor_copy(out=idx_tr[:, hs], in_=idx_tr_f[:, hs])

            # ---- the two big gathers (cost: ~1us desc-gen + ~0.4us transfer)
            nc.gpsimd.indirect_dma_start(
                out=emitv[:], out_offset=None, in_=em[:],
                in_offset=bass.IndirectOffsetOnAxis(ap=idx_em[:], axis=1),
            )
            nc.gpsimd.indirect_dma_start(
                out=transv[:], out_offset=None, in_=tmatf[:],
                in_offset=bass.IndirectOffsetOnAxis(ap=idx_tr[:], axis=1),
            )
            sv = sb.tile([P, 4], F32, name="sv")
            nc.gpsimd.indirect_dma_start(
                out=sv[:], out_offset=None, in_=startv[:],
                in_offset=bass.IndirectOffsetOnAxis(ap=sidx[:], axis=1),
            )

            # ---- end-term chain: seq lengths via ONE strided bf16 reduce
            # (0/1 inputs exact; DVE accumulates in f32), si math on Pool
            msq = sb.tile([P, 4], F32, name="msq")
            with nc.allow_low_precision(reason="0/1 bf16 mask, f32 accumulate"):
                nc.vector.tensor_reduce(
                    out=msq[:],
                    in_=mt2[:].rearrange("q (t u) -> q t u", t=4),
                    axis=mybir.AxisListType.X, op=AL.add,
                )
            # si = (msq - 1)*512 + b,  b = 128j + q
            si_f = sb.tile([P, 4], F32, name="si_f")
            nc.gpsimd.tensor_scalar(out=si_f[:], in0=msq[:], scalar1=512.0,
                                    scalar2=-512.0, op0=AL.mult, op1=AL.add)
            nc.gpsimd.tensor_scalar(out=si_f[:], in0=si_f[:], scalar1=iop[:],
                                    scalar2=None, op0=AL.add)
            nc.gpsimd.tensor_tensor(out=si_f[:], in0=si_f[:], in1=j128[:], op=AL.add)
            si4 = sb.tile([P, 4], I32, name="si4")
            nc.gpsimd.tensor_copy(out=si4[:], in_=si_f[:])
            lt = sb.tile([P, 4], I32, name="lt")
            nc.gpsimd.indirect_dma_start(
                out=lt[:], out_offset=None, in_=tagf_i[:],
                in_offset=bass.IndirectOffsetOnAxis(ap=si4[:], axis=1),
            )
            ev = sb.tile([P, 4], F32, name="ev")
            nc.gpsimd.indirect_dma_start(
                out=ev[:], out_offset=None, in_=endv[:],
                in_offset=bass.IndirectOffsetOnAxis(ap=lt[:], axis=1),
            )

            # ---- masks to [q, x] and epilogue
            memt, _ = to_qx("memt", mex_r, tag_pfx="ta")   # emission mask
            mtrt, _ = to_qx("mtrt", mtx_r, tag_pfx="tn")   # transition mask
            cs = sb.tile([P, NX], F32, name="cs")
            c1 = sb.tile([P, NX], F32, name="c1")
            nc.vector.tensor_tensor(out=c1[:], in0=emitv[:], in1=memt[:], op=AL.mult)
            nc.vector.tensor_tensor(out=cs[:], in0=transv[:], in1=mtrt[:], op=AL.mult)
            nc.vector.tensor_tensor(out=cs[:], in0=cs[:], in1=c1[:], op=AL.add)
            part = sb.tile([P, 4], F32, name="part")
            nc.vector.tensor_reduce(
                out=part[:],
                in_=cs[:].rearrange("p (u t) -> p t u", t=4),
                axis=mybir.AxisListType.X, op=AL.add,
            )

            # ---- total
            score = sb.tile([P, 4], F32, name="score")
            nc.vector.tensor_tensor(out=score[:], in0=part[:], in1=sv[:], op=AL.add)
            nc.vector.tensor_tensor(out=score[:], in0=score[:], in1=ev[:], op=AL.add)
            nc.sync.dma_start(out=out[:], in_=score[:])

    return nc


# ---------------------------------------------------------------------------
def _make_runner(nc, n_cores=8):
    import jax
    from jax.sharding import Mesh, PartitionSpec
    from jax.experimental.shard_map import shard_map
    import concourse.mybir as mybir
    from concourse import bass2jax

    bass2jax.install_neuronx_cc_hook()
    partition_name = nc.partition_id_tensor.name if nc.partition_id_tensor else None
    in_names, out_names, out_avals, zero_outs = [], [], [], []
    for alloc in nc.m.functions[0].allocations:
        if not isinstance(alloc, mybir.MemoryLocationSet):
            continue
        name = alloc.memorylocations[0].name
        if alloc.kind == "ExternalInput":
            if name != partition_name:
                in_names.append(name)
        elif alloc.kind == "ExternalOutput":
            shape = tuple(alloc.tensor_shape)
            dtype = mybir.dt.np(alloc.dtype)
            out_names.append(name)
            out_avals.append(jax.core.ShapedArray(shape, dtype))
            zero_outs.append(np.zeros(shape, dtype))
    n_params = len(in_names)
    all_in_names = list(in_names) + list(out_names)
    if partition_name is not None:
        all_in_names.append(partition_name)

    def _body(*args):
        operands = list(args)
        if partition_name is not None:
            operands.append(bass2jax.partition_id_tensor())
        outs = bass2jax._bass_exec_p.bind(
            *operands, out_avals=tuple(out_avals), in_names=tuple(all_in_names),
            out_names=tuple(out_names), lowering_input_output_aliases=(),
            sim_require_finite=True, sim_require_nnan=True, nc=nc,
        )
        return tuple(outs)

    devices = jax.devices()[:n_cores]
    mesh = Mesh(np.asarray(devices), ("core",))
    n_outs = len(out_names)
    jitted = jax.jit(
        shard_map(_body, mesh=mesh,
                  in_specs=(PartitionSpec("core"),) * (n_params + n_outs),
                  out_specs=(PartitionSpec("core"),) * n_outs, check_rep=False),
        keep_unused=True,
    )

    def run(in_maps):
        per_core = [[np.asarray(m[nm]) for nm in in_names] for m in in_maps]
        concat_in = [np.concatenate([per_core[c][i] for c in range(n_cores)], axis=0)
                     for i in range(n_params)]
        concat_zero = [np.concatenate([z] * n_cores, axis=0) for z in zero_outs]
        outs = [np.asarray(o) for o in jitted(*concat_in, *concat_zero)]
        results = []
        for c in range(n_cores):
            d = {}
            for i, nm in enumerate(out_names):
                per = outs[i].shape[0] // n_cores
                d[nm] = outs[i][c * per:(c + 1) * per]
            results.append(d)
        return results

    return run


def _get_runner():
    global _RUNNER
    if _RUNNER is None:
        _install_tile_patch()
        _RUNNER = _make_runner(_build_nc(), NCORES)
    return _RUNNER


# ---------------------------------------------------------------------------
def make_in_maps(emissions, tags, mask, start_transitions, end_transitions,
                 transitions):
    import ml_dtypes

    emissions = np.ascontiguousarray(emissions, dtype=np.float32)
    tags_i = np.asarray(tags).astype(np.int32)
    mask = np.ascontiguousarray(mask, dtype=np.int32)
    tmatf = np.ascontiguousarray(transitions, np.float32).reshape(1, -1)
    startv = np.ascontiguousarray(start_transitions, np.float32).reshape(1, NTAGS)
    endv = np.ascontiguousarray(end_transitions, np.float32).reshape(1, NTAGS)
    maskt_bf = np.ascontiguousarray(mask.T).astype(ml_dtypes.bfloat16).reshape(-1)
    tagf_i = tags_i.reshape(1, -1)
    zero128 = np.zeros((1, NTAGS), np.float32)

    in_maps = []
    for k in range(NCORES):
        s0 = k * SLICE
        em_k = emissions[s0:s0 + SLICE].reshape(1, -1)
        tag_k = np.ascontiguousarray(tags_i[s0:s0 + SLICE]).reshape(-1)
        if k < NCORES - 1:
            tagn_k = np.ascontiguousarray(tags_i[s0 + 1:s0 + SLICE + 1]).reshape(-1)
            masktr_k = np.ascontiguousarray(mask[s0 + 1:s0 + SLICE + 1]).reshape(-1)
        else:
            tagn_k = np.ascontiguousarray(
                np.concatenate([tags_i[s0 + 1:], tags_i[-1:]])).reshape(-1)
            masktr_k = np.concatenate(
                [mask[s0 + 1:], np.zeros((1, BATCH), np.int32)]).reshape(-1)
        maskem_k = mask[s0:s0 + SLICE].copy()
        if k == 0:
            maskem_k[0, :] = 1
        in_maps.append({
            "em": em_k,
            "tagx_i": tag_k,
            "tagnx_i": tagn_k,
            "maskem_i": maskem_k.reshape(-1),
            "masktr_i": np.ascontiguousarray(masktr_k, np.int32),
            "tmatf": tmatf,
            "startv": startv if k == 0 else zero128,
            "endv": endv if k == NCORES - 1 else zero128,
            "maskt_bf": maskt_bf,
            "tagf_i": tagf_i,
        })
    return in_maps


def kernel(emissions, tags, mask, start_transitions, end_transitions,
           transitions):
    run = _get_runner()
    in_maps = make_in_maps(emissions, tags, mask, start_transitions,
                           end_transitions, transitions)
    results = run(in_maps)
    total = np.zeros((P, 4), np.float64)
    for r in results:
        total += r["out"].astype(np.float64)
    score = total.T.reshape(BATCH).astype(np.float32)
    return score
